# revision 7
# baseline (speedup 1.0000x reference)
"""Trainium2 Bass kernel for CenterOfMass2DExtractor.

Full input x: (8, 4, 256, 256, 64) float32.  Output: (8, 4, 64) complex64
  mass[b,f,z]   = sum_{i,j} x[b,f,i,j,z]
  real[b,f,z]   = sum_{i,j} j * x / mass      (j = column index)
  imag[b,f,z]   = sum_{i,j} i * x / mass      (i = row index)

Sharding: pure data parallel over the batch dim -> 1 batch per NeuronCore
(8 cores), 64 MiB each, no communication.

Per-core kernel: view the shard as (f=4, t=128, p=128, v=256) where
t indexes blocks of 512 pixels (2 image rows), partition p holds 4
consecutive pixels q=0..3 (v = q*64 + z).  For each t: one 512 KiB DMA
(all 4 f), then 4 matmuls (one per q) with a 3-column stationary weight
  w[p, :] = [1, (4p+q) % 256, 2t + (p >= 64)]  =  [1, j, i]
and moving operand (p, f, z) = 256 columns in float32r (full-rate fp32),
accumulating [mass, sum j*x, sum i*x] into a single (3, 4, 64) PSUM tile
across all 512 matmuls.  The tiny (3, 256) result is copied to SBUF and
DMA'd out; the divide by mass and complex assembly happen on host.
"""

import numpy as np

_CACHE: dict = {}

NB, NF, NX, NY, NZ = 8, 4, 256, 256, 64
NT = 128          # t-blocks per f (512 pixels each)
NP = 128          # partitions
NV = 256          # values per partition per t-block (4 pixels * 64 z)


def _weights() -> np.ndarray:
    """(p, t, q, c) weight table: c = [mass, j, i]."""
    p = np.arange(NP).reshape(NP, 1, 1)
    t = np.arange(NT).reshape(1, NT, 1)
    q = np.arange(4).reshape(1, 1, 4)
    w = np.empty((NP, NT, 4, 3), np.float32)
    w[..., 0] = 1.0
    w[..., 1] = (4 * p + q) % NY
    w[..., 2] = 2 * t + (p >= 64)
    return w


def _build():
    import base64
    import io

    import concourse.bacc as bacc
    import concourse.bass as bass
    import concourse.mybir as mybir
    import concourse.tile as tile

    F32 = mybir.dt.float32
    F32R = mybir.dt.float32r

    nc = bacc.Bacc("TRN2", target_bir_lowering=False)
    x_dram = nc.dram_tensor("x", [NF, NT, NP, NV], F32R, kind="ExternalInput")
    out_dram = nc.dram_tensor("out", [3, NF * NZ], F32, kind="ExternalOutput")

    # inline const weight table, declared float32r (bytes are plain fp32)
    W = _weights()
    mls = nc._tensor("w", list(W.shape), F32R, kind="Const", type="DRAM")
    buf = io.BytesIO()
    np.save(buf, W, allow_pickle=False)
    mls.file = "w.npy"
    mls.ant_data = base64.standard_b64encode(buf.getvalue()).decode()
    w_dram = bass.DRamTensorHandle("w", list(W.shape), F32R)

    with tile.TileContext(nc) as tc:
        with (
            tc.tile_pool(name="const", bufs=1) as const_pool,
            tc.tile_pool(name="xp", bufs=6) as xpool,
            tc.tile_pool(name="op", bufs=1) as opool,
            tc.tile_pool(name="ps", bufs=1, space=bass.MemorySpace.PSUM) as pspool,
        ):
            w_tile = const_pool.tile([NP, NT, 4, 3], F32R)
            nc.sync.dma_start(out=w_tile[:], in_=w_dram[:])

            acc = pspool.tile([3, NF, NZ], F32)

            for t in range(NT):
                xt = xpool.tile([NP, NF, 4, NZ], F32R)  # [p, f, q, z]
                # one DMA per t: per partition, 4 chunks of 1 KiB (one per f)
                nc.sync.dma_start(
                    out=xt[:],
                    in_=x_dram[:, t, :, :].rearrange("f p v -> p f v"),
                )
                for q in range(4):
                    nc.tensor.matmul(
                        acc[:],
                        lhsT=w_tile[:, t, q, :],
                        rhs=xt[:, :, q, :],
                        start=(t == 0 and q == 0),
                        stop=(t == NT - 1 and q == 3),
                    )

            res = opool.tile([3, NF * NZ], F32)
            nc.vector.tensor_copy(out=res[:], in_=acc[:].rearrange("c f z -> c (f z)"))
            nc.sync.dma_start(out=out_dram[:], in_=res[:])

    nc.compile()
    return nc


def _get_nc():
    if "nc" not in _CACHE:
        _CACHE["nc"] = _build()
    return _CACHE["nc"]


def kernel(x: np.ndarray) -> np.ndarray:
    from concourse.bass_utils import run_bass_kernel_spmd

    x = np.asarray(x)
    assert x.shape == (NB, NF, NX, NY, NZ), x.shape
    in_dtype = x.dtype
    x = np.ascontiguousarray(x, dtype=np.float32)

    nc = _get_nc()
    in_maps = [{"x": x[b].reshape(NF, NT, NP, NV)} for b in range(NB)]
    results = run_bass_kernel_spmd(nc, in_maps, core_ids=list(range(NB))).results

    out = np.empty((NB, NF, NZ), np.complex64)
    for b in range(NB):
        sums = np.asarray(results[b]["out"]).reshape(3, NF, NZ)
        mass = sums[0]
        out[b] = (sums[1] / mass + 1j * (sums[2] / mass)).astype(np.complex64)
    del in_dtype
    return out
